# revision 1
# baseline (speedup 1.0000x reference)
"""Trainium2 Bass kernel for nn_BaseTransformer (B=16, C=128, L=1024, H=8, dk=dv=32).

Sharding: pure data-parallel over batch — 8 cores x 2 batches each, no collectives.

Per-core algorithm (PE datapath in bf16 — fp32 matmuls stream at 1/4 rate on
this PE; PSUM accumulation and softmax normalization stay fp32):
  - QK projection: chunks of rows [q h0-3 | q h4-7 | k h0-3 | k h4-7], SCALE and
    q-bias folded in host-side (k bias dropped: softmax-invariant; v bias folded
    into the output bias via W_o @ b_v since softmax rows sum to 1).
  - v is projected TRANSPOSED (x^T @ Wv^T) so the PV matmul needs no transposes.
  - logits are computed transposed (S^T[t,s]) so softmax reduction happens via
    matmul against an all-ones stationary (denominator replicated over each
    head's 32 output partitions); exp runs on ScalarE straight out of PSUM.
  - attention matmuls are packed with tile_position (row-packing for K=32 QK,
    col-packing for M=32 PV and denominator) to use more of the PE array.
  - All compute ops keep out/in0/in1 at identical base partitions.
"""

import os
import numpy as np

B, C, L = 16, 128, 1024
DK, DV, H = 32, 32, 8
SCALE = DK ** (-0.5)
NCORES = 8
BLOC = B // NCORES  # batches per core

_CACHE = {}

# bisect stages: proj < qkexp < pv < norm < full
_STAGES = ["proj", "qkexp", "pv", "norm", "full"]


def _stage():
    return os.environ.get("KSTAGE", "full")


def _stage_ge(s):
    return _STAGES.index(_stage()) >= _STAGES.index(s)


def _split_excess_waits(nc, mybir, cap=1):
    """This container's walrus rejects instructions carrying more than one
    sync-wait command ("Too many sync wait commands" in setupSyncWait), while
    Tile freely attaches several. Move all but `cap` waits of every
    instruction onto injected same-engine NoOps placed immediately before it
    (same block order == same engine queue order, so semantics are identical:
    all waits still complete before the instruction issues)."""
    ctr = 0
    for f in nc.m.functions:
        for blk in f.blocks:
            out = []
            changed = False
            for ins in blk.instructions:
                si = ins.sync_info
                waits = list(si.on_wait) if si and si.on_wait else []
                eng = getattr(ins, "engine", None)
                if len(waits) > cap and eng is not None:
                    for w in waits[:-cap]:
                        nop = mybir.InstNoOp(name=f"I-wsplit-{ctr}")
                        ctr += 1
                        nop.engine = eng
                        nop.sync_info = mybir.SyncInfo(on_wait=[w], on_update=[])
                        out.append(nop)
                    ins.sync_info = mybir.SyncInfo(
                        on_wait=waits[-cap:], on_update=list(si.on_update or [])
                    )
                    changed = True
                out.append(ins)
            if changed:
                blk.instructions = out


def _build_nc():
    import concourse.bass as bass
    import concourse.tile as tile
    from concourse import mybir
    from contextlib import ExitStack

    f32 = mybir.dt.float32
    bf16 = mybir.dt.bfloat16
    nc = bass.Bass()

    x_d = nc.dram_tensor("x_sh", [BLOC, C, L], bf16, kind="ExternalInput")
    wqk_d = nc.dram_tensor("wqk", [C, 4, 128], bf16, kind="ExternalInput")
    bqk_d = nc.dram_tensor("bqk", [128, 2], f32, kind="ExternalInput")
    wv_d = nc.dram_tensor("wv", [C, 256], bf16, kind="ExternalInput")
    wo_d = nc.dram_tensor("wo", [128, 3, 128], bf16, kind="ExternalInput")
    bout_d = nc.dram_tensor("bout", [128, 1], f32, kind="ExternalInput")
    out_d = nc.dram_tensor("out_sh", [BLOC, C, L], f32, kind="ExternalOutput")

    Exp = mybir.ActivationFunctionType.Exp
    mult = mybir.AluOpType.mult

    with tile.TileContext(nc) as tc, ExitStack() as ctx:
        consts = ctx.enter_context(tc.tile_pool(name="consts", bufs=1))
        xp = ctx.enter_context(tc.tile_pool(name="xp", bufs=2))
        qkp = ctx.enter_context(tc.tile_pool(name="qkp", bufs=2))
        vtp = ctx.enter_context(tc.tile_pool(name="vtp", bufs=2))
        stp = ctx.enter_context(tc.tile_pool(name="stp", bufs=4))
        zfp = ctx.enter_context(tc.tile_pool(name="zfp", bufs=2))
        rbp = ctx.enter_context(tc.tile_pool(name="rbp", bufs=3))
        outp = ctx.enter_context(tc.tile_pool(name="outp", bufs=2))
        pbig = ctx.enter_context(tc.tile_pool(name="pbig", bufs=2, space="PSUM"))
        pacc = ctx.enter_context(tc.tile_pool(name="pacc", bufs=2, space="PSUM"))

        wqk_sb = consts.tile([C, 4, 128], bf16, name="wqk_sb")
        bqk_sb = consts.tile([128, 2], f32, name="bqk_sb")
        wv_sb = consts.tile([C, 256], bf16, name="wv_sb")
        wo_sb = consts.tile([128, 3, 128], bf16, name="wo_sb")
        bout_sb = consts.tile([128, 1], f32, name="bout_sb")
        ones_sb = consts.tile([128, 32], bf16, name="ones_sb")
        nc.sync.dma_start(out=wqk_sb, in_=wqk_d[:, :, :])
        nc.sync.dma_start(out=bqk_sb, in_=bqk_d[:, :])
        nc.sync.dma_start(out=wv_sb, in_=wv_d[:, :])
        nc.sync.dma_start(out=wo_sb, in_=wo_d[:, :, :])
        nc.sync.dma_start(out=bout_sb, in_=bout_d[:, :])
        nc.vector.memset(ones_sb, 1.0)

        denmerge = bool(int(os.environ.get("KDENMERGE", "1")))
        repeat = int(os.environ.get("KREPEAT", "1"))
        for _rep in range(repeat):
          S = {}
          def _proj(b):
            x_sb = xp.tile([C, L], bf16, name="x_sb")
            nc.sync.dma_start(out=x_sb, in_=x_d[b])

            # ---- QK projection: psum -> (q bias-add | k copy) -> SBUF
            qA = qkp.tile([128, L], bf16, name="qA")
            qB = qkp.tile([128, L], bf16, name="qB")
            kA = qkp.tile([128, L], bf16, name="kA")
            kB = qkp.tile([128, L], bf16, name="kB")
            tgts = [qA, qB, kA, kB]
            for cch in (0, 2, 1, 3):
                ps = pbig.tile([128, L], f32, name="pl")
                for jh in range(2):
                    nc.tensor.matmul(
                        out=ps[:, 512 * jh : 512 * jh + 512],
                        lhsT=wqk_sb[:, cch, :],
                        rhs=x_sb[:, 512 * jh : 512 * jh + 512],
                        start=True, stop=True,
                    )
                if cch < 2:
                    nc.vector.tensor_scalar_add(
                        out=tgts[cch], in0=ps, scalar1=bqk_sb[:, cch : cch + 1]
                    )
                else:
                    nc.vector.tensor_copy(out=tgts[cch], in_=ps)

            # ---- V^T projection: vt[t, i, h, d] = v_h[d, 128 i + t]
            # With KDENMERGE, each head's stationary is [v (32 cols) | ones
            # (32 cols)] so the PV matmul also produces the softmax
            # denominator (replicated over 32 partitions) in the same pass.
            vw = 64 if denmerge else 32
            vt = vtp.tile([128, 8, 8, vw], bf16, name="vt")
            for gq in range(2):
                ps = pbig.tile([128, L], f32, name="pl")
                for q in range(4):
                    nc.tensor.matmul(
                        out=ps[:, 256 * q : 256 * q + 256],
                        lhsT=x_sb[:, 128 * (4 * gq + q) : 128 * (4 * gq + q) + 128],
                        rhs=wv_sb,
                        start=True, stop=True,
                    )
                nc.vector.tensor_copy(
                    out=vt[:, 4 * gq : 4 * gq + 4, :, 0:32],
                    in_=ps.rearrange("p (a h d) -> p a h d", h=8, d=32),
                )
            if denmerge:
                nc.gpsimd.memset(vt[:, :, :, 32:64], 1.0)

            S[b] = dict(x_sb=x_sb, qA=qA, qB=qB, kA=kA, kB=kB, vt=vt)

          def _attn(b, g):
            x_sb, qA, qB, kA, kB, vt = (S[b][k] for k in
                ("x_sb", "qA", "qB", "kA", "kB", "vt"))
            if g == 0:
                S[b]["zfA"] = zfp.tile([128, L], bf16, name="zfA")
                S[b]["zfB"] = zfp.tile([128, L], bf16, name="zfB")
            zfA, zfB = S[b]["zfA"], S[b]["zfB"]
            if True:
                q_t = (qA, qB)[g]
                k_t = (kA, kB)[g]
                zf = (zfA, zfB)[g]
                for j in range(2):
                    sj = slice(512 * j, 512 * j + 512)
                    if denmerge:
                        # comb[p?]: per head pair: [z_even | den_even | z_odd | den_odd]
                        combs = [pacc.tile([128, 512], f32, name="comb", bufs=4)
                                 for _ in range(2)]
                    else:
                        zden = pacc.tile([128, 512], f32, name="zden")
                        denb = pacc.tile([128, 512], f32, name="denb")
                    for i in range(8):
                        sts = []
                        for pp in range(2):  # head pairs within group
                            r0 = 64 * pp
                            pl = pbig.tile([128, 2, 512], f32, name="pl")
                            for hh in range(2):
                                rr = r0 + 32 * hh
                                nc.tensor.matmul(
                                    out=pl[:, hh, :],
                                    lhsT=k_t[rr : rr + 32, 128 * i : 128 * i + 128],
                                    rhs=q_t[rr : rr + 32, sj],
                                    start=True, stop=True,
                                    tile_position=(rr, 0),
                                )
                            st = stp.tile([128, 2, 512], bf16, name="st")
                            nc.scalar.activation(out=st, in_=pl, func=Exp)
                            sts.append(st)
                        if denmerge:
                            for pp in range(2):
                                for hh in range(2):
                                    nc.tensor.matmul(
                                        out=combs[pp][64 * hh : 64 * hh + 64, :],
                                        lhsT=vt[:, i, 4 * g + 2 * pp + hh, :],
                                        rhs=sts[pp][:, hh, :],
                                        start=(i == 0), stop=(i == 7),
                                        tile_position=(0, 64 * hh),
                                        skip_group_check=True,
                                    )
                        else:
                            for hl in range(4):  # head-local index in group
                                st = sts[hl // 2]
                                mv = st[:, hl % 2, :]
                                nc.tensor.matmul(
                                    out=zden[32 * hl : 32 * hl + 32, :],
                                    lhsT=vt[:, i, 4 * g + hl, :],
                                    rhs=mv,
                                    start=(i == 0), stop=(i == 7),
                                    tile_position=(0, 32 * hl),
                                    skip_group_check=True,
                                )
                                nc.tensor.matmul(
                                    out=denb[32 * hl : 32 * hl + 32, :],
                                    lhsT=ones_sb,
                                    rhs=mv,
                                    start=(i == 0), stop=(i == 7),
                                    tile_position=(0, 32 * hl),
                                    skip_group_check=True,
                                )
                    if denmerge:
                        for pp in range(2):
                            rb = rbp.tile([128, 512], f32, name="rb")
                            nc.vector.reciprocal(out=rb, in_=combs[pp])
                            for hh in range(2):
                                h4 = (2 * pp + hh) % 4
                                nc.vector.tensor_tensor(
                                    out=zf[32 * h4 : 32 * h4 + 32, sj],
                                    in0=combs[pp][64 * hh : 64 * hh + 32, :],
                                    in1=rb[64 * hh + 32 : 64 * hh + 64, :],
                                    op=mult,
                                )
                    elif _stage_ge("norm"):
                        rb = rbp.tile([128, 512], f32, name="rb")
                        nc.vector.reciprocal(out=rb, in_=denb)
                        nc.vector.tensor_tensor(
                            out=zf[:, sj], in0=zden, in1=rb, op=mult
                        )
                    else:
                        nc.vector.tensor_copy(out=zf[:, sj], in_=zden)

          def _wo(b):
            x_sb, zfA, zfB = (S[b][k] for k in ("x_sb", "zfA", "zfB"))
            # ---- output projection + residual projection + bias
            po = pbig.tile([128, L], f32, name="pl")
            for j in range(2):
                sj = slice(512 * j, 512 * j + 512)
                nc.tensor.matmul(out=po[:, sj], lhsT=wo_sb[:, 0, :], rhs=zfA[:, sj],
                                 start=True, stop=False)
                nc.tensor.matmul(out=po[:, sj], lhsT=wo_sb[:, 1, :], rhs=zfB[:, sj],
                                 start=False, stop=False)
                nc.tensor.matmul(out=po[:, sj], lhsT=wo_sb[:, 2, :], rhs=x_sb[:, sj],
                                 start=False, stop=True)
            o_sb = outp.tile([128, L], f32, name="o_sb")
            nc.vector.tensor_scalar_add(out=o_sb, in0=po, scalar1=bout_sb[:, 0:1])
            nc.sync.dma_start(out=out_d[b], in_=o_sb)

          # Interleaved schedule: batch b+1's projections are emitted between
          # batch b's two attention head-groups so they execute on the PE FIFO
          # while ScalarE is still busy with batch b's exps; W_o is inline so
          # only the last batch's output tail is unoverlapped.
          for b in range(BLOC):
            _proj(b)
          for b in range(BLOC):
            _attn(b, 0)
            _attn(b, 1)
          for b in range(BLOC):
            _wo(b)

    _split_excess_waits(nc, mybir)
    nc.finalize()
    return nc


def get_nc():
    if "nc" not in _CACHE:
        _CACHE["nc"] = _build_nc()
    return _CACHE["nc"]


def prep_weights(w_qkv, b_qkv, w_o, b_o, w_res, b_res):
    w_qkv = np.asarray(w_qkv, np.float32)
    b_qkv = np.asarray(b_qkv, np.float32)
    w_o = np.asarray(w_o, np.float32)
    b_o = np.asarray(b_o, np.float32)
    w_res = np.asarray(w_res, np.float32)
    b_res = np.asarray(b_res, np.float32)

    d = np.arange(32)
    qrows = np.concatenate([96 * h + d for h in range(H)])        # (256,)
    krows = np.concatenate([96 * h + 32 + d for h in range(H)])
    vrows = np.concatenate([96 * h + 64 + d for h in range(H)])

    Wq = w_qkv[qrows] * SCALE                                     # (256, C)
    Wk = w_qkv[krows]
    wqk = np.stack([Wq[:128].T, Wq[128:].T, Wk[:128].T, Wk[128:].T], axis=1)
    bqk = np.stack([b_qkv[qrows[:128]], b_qkv[qrows[128:]]], axis=1) * SCALE
    wv = np.ascontiguousarray(w_qkv[vrows].T)                     # (C, 256)
    wo = np.stack([w_o[:, :128].T, w_o[:, 128:].T, w_res.T], axis=1)
    bv = b_qkv[vrows]
    bout = (b_o + b_res + w_o @ bv)[:, None]

    import ml_dtypes
    bf = ml_dtypes.bfloat16
    return {
        "wqk": np.ascontiguousarray(wqk, bf),
        "bqk": np.ascontiguousarray(bqk, np.float32),
        "wv": np.ascontiguousarray(wv, bf),
        "wo": np.ascontiguousarray(wo, bf),
        "bout": np.ascontiguousarray(bout, np.float32),
    }


def make_in_maps(x, weights):
    import ml_dtypes
    x = np.ascontiguousarray(np.asarray(x).astype(ml_dtypes.bfloat16))
    return [
        dict(x_sh=np.ascontiguousarray(x[BLOC * i : BLOC * i + BLOC]), **weights)
        for i in range(NCORES)
    ]


class Runner:
    """Persistent PJRT executable for the SPMD bass program (axon path).

    Mirrors concourse.bass2jax.run_bass_via_pjrt's multi-core branch, but keeps
    the jitted callable so repeated executions don't re-trace/re-compile —
    needed both for a fast kernel() and for timing loops in test.py.
    """

    def __init__(self, nc=None, donate=True):
        import jax
        import concourse.mybir as mybir
        from concourse import bass2jax
        from jax.experimental.shard_map import shard_map
        from jax.sharding import Mesh, PartitionSpec

        if nc is None:
            nc = get_nc()
        bass2jax.install_neuronx_cc_hook()

        in_names, out_names, out_avals = [], [], []
        partition_name = (
            nc.partition_id_tensor.name if nc.partition_id_tensor else None
        )
        for alloc in nc.m.functions[0].allocations:
            if not isinstance(alloc, mybir.MemoryLocationSet):
                continue
            name = alloc.memorylocations[0].name
            if alloc.kind == "ExternalInput":
                if name != partition_name:
                    in_names.append(name)
            elif alloc.kind == "ExternalOutput":
                shape = tuple(alloc.tensor_shape)
                dtype = mybir.dt.np(alloc.dtype)
                out_avals.append(jax.core.ShapedArray(shape, dtype))
                out_names.append(name)
        n_params = len(in_names)
        n_outs = len(out_avals)
        all_in_names = list(in_names) + list(out_names)
        if partition_name is not None:
            all_in_names.append(partition_name)
        self.in_names = in_names
        self.out_names = out_names
        self.out_avals = out_avals

        donate_idx = tuple(range(n_params, n_params + n_outs)) if donate else ()

        def _body(*args):
            operands = list(args)
            if partition_name is not None:
                operands.append(bass2jax.partition_id_tensor())
            outs = bass2jax._bass_exec_p.bind(
                *operands,
                out_avals=tuple(out_avals),
                in_names=tuple(all_in_names),
                out_names=tuple(out_names),
                lowering_input_output_aliases=(),
                sim_require_finite=True,
                sim_require_nnan=True,
                nc=nc,
            )
            return tuple(outs)

        devices = jax.devices()[:NCORES]
        assert len(devices) == NCORES
        mesh = Mesh(np.asarray(devices), ("core",))
        in_specs = (PartitionSpec("core"),) * (n_params + n_outs)
        out_specs = (PartitionSpec("core"),) * n_outs
        self.sharded = jax.jit(
            shard_map(_body, mesh=mesh, in_specs=in_specs, out_specs=out_specs,
                      check_rep=False),
            donate_argnums=donate_idx,
            keep_unused=True,
        )
        self.mesh = mesh

    def prep(self, in_maps):
        return [
            np.concatenate([np.asarray(m[name]) for m in in_maps], axis=0)
            for name in self.in_names
        ]

    def zeros(self):
        return [
            np.zeros((NCORES * a.shape[0], *a.shape[1:]), a.dtype)
            for a in self.out_avals
        ]

    def call_async(self, concat_in):
        return self.sharded(*concat_in, *self.zeros())

    def __call__(self, in_maps):
        outs = self.call_async(self.prep(in_maps))
        arr = np.asarray(outs[0])
        return arr.reshape(NCORES, *self.out_avals[0].shape)


def get_runner():
    if "runner" not in _CACHE:
        _CACHE["runner"] = Runner()
    return _CACHE["runner"]


def run(x, weights, **kw):
    runner = get_runner()
    per_core = runner(make_in_maps(x, weights))
    out = per_core.reshape(B, C, L)
    return out, None


def kernel(x, w_qkv, b_qkv, w_o, b_o, w_res, b_res):
    weights = prep_weights(w_qkv, b_qkv, w_o, b_o, w_res, b_res)
    out, _ = run(x, weights)
    return out



# revision 7
# speedup vs baseline: 3.9716x; 3.9716x over previous
"""Trainium2 Bass kernel for nn_BaseTransformer (B=16, C=128, L=1024, H=8, dk=dv=32).

Sharding: pure data-parallel over batch — 8 cores x 2 batches each, no collectives.

Per-core algorithm (PE datapath in bf16 — fp32 matmuls stream at 1/4 rate on
this PE; PSUM accumulation and softmax normalization stay fp32):
  - QK projection: chunks of rows [q h0-3 | q h4-7 | k h0-3 | k h4-7], SCALE and
    q-bias folded in host-side (k bias dropped: softmax-invariant; v bias folded
    into the output bias via W_o @ b_v since softmax rows sum to 1).
  - v is projected TRANSPOSED (x^T @ Wv^T) so the PV matmul needs no transposes.
  - logits are computed transposed (S^T[t,s]) so softmax reduction happens via
    matmul against an all-ones stationary (denominator replicated over each
    head's 32 output partitions); exp runs on ScalarE straight out of PSUM.
  - attention matmuls are packed with tile_position (row-packing for K=32 QK,
    col-packing for M=32 PV and denominator) to use more of the PE array.
  - All compute ops keep out/in0/in1 at identical base partitions.

The exp of the logits (16.8M elements/core) is the critical path: every
element crosses PSUM->SBUF through ScalarE (1 elem/cycle @1.2GHz) or DVE
(1 elem/cycle @0.96GHz for fp32 src), so the softmax exponentials are SPLIT
across both engines: ScalarE runs exact Exp on one head-pair tile while DVE
runs a Schraudolph fast-exp on the other — tensor_scalar computing
int16(round(s*128/ln2 + (127-c)*128)) whose int16 bits reinterpreted as bf16
are e^s with ~3% max rel error (softmax ratios cancel most of it; end-to-end
rel err stays ~2.6e-3). PSUM->SBUF copies of q/k/v projections also run on
ScalarE (activation-with-bias for q) to balance engine load.
"""

import os
import numpy as np

B, C, L = 16, 128, 1024
DK, DV, H = 32, 32, 8
SCALE = DK ** (-0.5)
NCORES = 8
BLOC = B // NCORES  # batches per core

_CACHE = {}

# bisect stages: proj < qkexp < pv < norm < full
_STAGES = ["proj", "qkexp", "pv", "norm", "full"]


def _stage():
    return os.environ.get("KSTAGE", "full")


def _stage_ge(s):
    return _STAGES.index(_stage()) >= _STAGES.index(s)


def _split_excess_waits(nc, mybir, cap=1):
    """This container's walrus rejects instructions carrying more than one
    sync-wait command ("Too many sync wait commands" in setupSyncWait), while
    Tile freely attaches several. Move all but `cap` waits of every
    instruction onto injected same-engine NoOps placed immediately before it
    (same block order == same engine queue order, so semantics are identical:
    all waits still complete before the instruction issues)."""
    ctr = 0
    for f in nc.m.functions:
        for blk in f.blocks:
            out = []
            changed = False
            for ins in blk.instructions:
                si = ins.sync_info
                waits = list(si.on_wait) if si and si.on_wait else []
                eng = getattr(ins, "engine", None)
                if len(waits) > cap and eng is not None:
                    for w in waits[:-cap]:
                        nop = mybir.InstNoOp(name=f"I-wsplit-{ctr}")
                        ctr += 1
                        nop.engine = eng
                        nop.sync_info = mybir.SyncInfo(on_wait=[w], on_update=[])
                        out.append(nop)
                    ins.sync_info = mybir.SyncInfo(
                        on_wait=waits[-cap:], on_update=list(si.on_update or [])
                    )
                    changed = True
                out.append(ins)
            if changed:
                blk.instructions = out


def _build_nc():
    import concourse.bass as bass
    import concourse.tile as tile
    from concourse import mybir
    from contextlib import ExitStack

    f32 = mybir.dt.float32
    bf16 = mybir.dt.bfloat16
    nc = bass.Bass()

    x_d = nc.dram_tensor("x_sh", [BLOC, C, L], bf16, kind="ExternalInput")
    wqk_d = nc.dram_tensor("wqk", [C, 4, 128], bf16, kind="ExternalInput")
    bqk_d = nc.dram_tensor("bqk", [128, 2], f32, kind="ExternalInput")
    wv_d = nc.dram_tensor("wv", [C, 256], bf16, kind="ExternalInput")
    wo_d = nc.dram_tensor("wo", [128, 3, 128], bf16, kind="ExternalInput")
    bout_d = nc.dram_tensor("bout", [128, 1], f32, kind="ExternalInput")
    out_d = nc.dram_tensor("out_sh", [BLOC, C, L], f32, kind="ExternalOutput")

    Exp = mybir.ActivationFunctionType.Exp
    Ident = mybir.ActivationFunctionType.Identity
    mult = mybir.AluOpType.mult
    add = mybir.AluOpType.add
    i16 = mybir.dt.int16

    # Schraudolph fast-exp targeting bf16 bits via int16:
    #   int16(s*128/ln2 + (127 - C)*128) bitcast to bf16 ~= e^s
    FEXP_A = 128.0 / float(np.log(2.0))
    FEXP_C = float(os.environ.get("KFEXPC", "0.0435"))
    FEXP_B = 128.0 * (127.0 - FEXP_C)

    with tile.TileContext(nc) as tc, ExitStack() as ctx:
        consts = ctx.enter_context(tc.tile_pool(name="consts", bufs=1))
        xp = ctx.enter_context(tc.tile_pool(name="xp", bufs=2))
        qkp = ctx.enter_context(tc.tile_pool(name="qkp", bufs=2))
        vtp = ctx.enter_context(tc.tile_pool(name="vtp", bufs=2))
        stp = ctx.enter_context(tc.tile_pool(name="stp", bufs=4))
        zfp = ctx.enter_context(tc.tile_pool(name="zfp", bufs=2))
        rbp = ctx.enter_context(tc.tile_pool(name="rbp", bufs=3))
        outp = ctx.enter_context(tc.tile_pool(name="outp", bufs=2))
        pbig = ctx.enter_context(tc.tile_pool(name="pbig", bufs=2, space="PSUM"))
        pacc = ctx.enter_context(tc.tile_pool(name="pacc", bufs=2, space="PSUM"))

        wqk_sb = consts.tile([C, 4, 128], bf16, name="wqk_sb")
        bqk_sb = consts.tile([128, 2], f32, name="bqk_sb")
        wv_sb = consts.tile([C, 256], bf16, name="wv_sb")
        wo_sb = consts.tile([128, 3, 128], bf16, name="wo_sb")
        bout_sb = consts.tile([128, 1], f32, name="bout_sb")
        ones_sb = consts.tile([128, 32], bf16, name="ones_sb")
        nc.sync.dma_start(out=wqk_sb, in_=wqk_d[:, :, :])
        nc.sync.dma_start(out=bqk_sb, in_=bqk_d[:, :])
        nc.sync.dma_start(out=wv_sb, in_=wv_d[:, :])
        nc.sync.dma_start(out=wo_sb, in_=wo_d[:, :, :])
        nc.sync.dma_start(out=bout_sb, in_=bout_d[:, :])
        nc.vector.memset(ones_sb, 1.0)

        denmerge = bool(int(os.environ.get("KDENMERGE", "0")))
        vexp = bool(int(os.environ.get("KVEXP", "1")))
        # KXTRA>0: that many i-iters per (g,j) send the DVE tile to ScalarE
        # instead; KXTRA<0: ScalarE's tile goes to DVE. Load-balance knob.
        xtra = int(os.environ.get("KXTRA", "0"))
        repeat = int(os.environ.get("KREPEAT", "1"))
        for _rep in range(repeat):
          S = {}
          def _proj(b):
            x_sb = xp.tile([C, L], bf16, name="x_sb")
            nc.sync.dma_start(out=x_sb, in_=x_d[b])

            # ---- QK projection: psum -> (q bias-add | k copy) -> SBUF
            qA = qkp.tile([128, L], bf16, name="qA")
            qB = qkp.tile([128, L], bf16, name="qB")
            kA = qkp.tile([128, L], bf16, name="kA")
            kB = qkp.tile([128, L], bf16, name="kB")
            tgts = [qA, qB, kA, kB]
            for cch in (0, 2, 1, 3):
                ps = pbig.tile([128, L], f32, name="pl")
                for jh in range(2):
                    nc.tensor.matmul(
                        out=ps[:, 512 * jh : 512 * jh + 512],
                        lhsT=wqk_sb[:, cch, :],
                        rhs=x_sb[:, 512 * jh : 512 * jh + 512],
                        start=True, stop=True,
                    )
                if cch < 2:
                    nc.scalar.activation(
                        out=tgts[cch], in_=ps, func=Ident,
                        bias=bqk_sb[:, cch : cch + 1],
                    )
                else:
                    nc.scalar.copy(out=tgts[cch], in_=ps)

            # ---- V^T projection: vt[t, i, h, d] = v_h[d, 128 i + t]
            # With KDENMERGE, each head's stationary is [v (32 cols) | ones
            # (32 cols)] so the PV matmul also produces the softmax
            # denominator (replicated over 32 partitions) in the same pass.
            vw = 64 if denmerge else 32
            vt = vtp.tile([128, 8, 8, vw], bf16, name="vt")
            for gq in range(2):
                ps = pbig.tile([128, L], f32, name="pl")
                for q in range(4):
                    nc.tensor.matmul(
                        out=ps[:, 256 * q : 256 * q + 256],
                        lhsT=x_sb[:, 128 * (4 * gq + q) : 128 * (4 * gq + q) + 128],
                        rhs=wv_sb,
                        start=True, stop=True,
                    )
                nc.scalar.copy(
                    out=vt[:, 4 * gq : 4 * gq + 4, :, 0:32],
                    in_=ps.rearrange("p (a h d) -> p a h d", h=8, d=32),
                )
            if denmerge:
                nc.gpsimd.memset(vt[:, :, :, 32:64], 1.0)

            S[b] = dict(x_sb=x_sb, qA=qA, qB=qB, kA=kA, kB=kB, vt=vt)

          def _attn(b, g):
            x_sb, qA, qB, kA, kB, vt = (S[b][k] for k in
                ("x_sb", "qA", "qB", "kA", "kB", "vt"))
            if g == 0:
                S[b]["zfA"] = zfp.tile([128, L], bf16, name="zfA")
                S[b]["zfB"] = zfp.tile([128, L], bf16, name="zfB")
            zfA, zfB = S[b]["zfA"], S[b]["zfB"]
            if True:
                q_t = (qA, qB)[g]
                k_t = (kA, kB)[g]
                zf = (zfA, zfB)[g]
                for j in range(2):
                    sj = slice(512 * j, 512 * j + 512)
                    if denmerge:
                        # comb[p?]: per head pair: [z_even | den_even | z_odd | den_odd]
                        combs = [pacc.tile([128, 512], f32, name="comb", bufs=4)
                                 for _ in range(2)]
                    else:
                        zden = pacc.tile([128, 512], f32, name="zden")
                        denb = pacc.tile([128, 512], f32, name="denb")
                    for i in range(8):
                        sts = []
                        for pp in range(2):  # head pairs within group
                            r0 = 64 * pp
                            pl = pbig.tile([128, 2, 512], f32, name="pl")
                            for hh in range(2):
                                rr = r0 + 32 * hh
                                nc.tensor.matmul(
                                    out=pl[:, hh, :],
                                    lhsT=k_t[rr : rr + 32, 128 * i : 128 * i + 128],
                                    rhs=q_t[rr : rr + 32, sj],
                                    start=True, stop=True,
                                    tile_position=(rr, 0),
                                )
                            st = stp.tile([128, 2, 512], bf16, name="st")
                            if vexp:
                                on_dve = (pp == 1 and i >= xtra) or (
                                    pp == 0 and xtra < 0 and i < -xtra
                                )
                            else:
                                on_dve = False
                            if on_dve:
                                nc.vector.tensor_scalar(
                                    out=st.bitcast(i16)[:, :, :], in0=pl,
                                    scalar1=FEXP_A, scalar2=FEXP_B,
                                    op0=mult, op1=add,
                                )
                            else:
                                nc.scalar.activation(out=st, in_=pl, func=Exp)
                            sts.append(st)
                        if denmerge:
                            for pp in range(2):
                                for hh in range(2):
                                    nc.tensor.matmul(
                                        out=combs[pp][64 * hh : 64 * hh + 64, :],
                                        lhsT=vt[:, i, 4 * g + 2 * pp + hh, :],
                                        rhs=sts[pp][:, hh, :],
                                        start=(i == 0), stop=(i == 7),
                                        tile_position=(0, 64 * hh),
                                        skip_group_check=True,
                                    )
                        else:
                            for hl in range(4):  # head-local index in group
                                st = sts[hl // 2]
                                mv = st[:, hl % 2, :]
                                nc.tensor.matmul(
                                    out=zden[32 * hl : 32 * hl + 32, :],
                                    lhsT=vt[:, i, 4 * g + hl, :],
                                    rhs=mv,
                                    start=(i == 0), stop=(i == 7),
                                    tile_position=(0, 32 * hl),
                                    skip_group_check=True,
                                )
                                nc.tensor.matmul(
                                    out=denb[32 * hl : 32 * hl + 32, :],
                                    lhsT=ones_sb,
                                    rhs=mv,
                                    start=(i == 0), stop=(i == 7),
                                    tile_position=(0, 32 * hl),
                                    skip_group_check=True,
                                )
                    if denmerge:
                        for pp in range(2):
                            rb = rbp.tile([128, 512], f32, name="rb")
                            nc.vector.reciprocal(out=rb, in_=combs[pp])
                            for hh in range(2):
                                h4 = (2 * pp + hh) % 4
                                nc.vector.tensor_tensor(
                                    out=zf[32 * h4 : 32 * h4 + 32, sj],
                                    in0=combs[pp][64 * hh : 64 * hh + 32, :],
                                    in1=rb[64 * hh + 32 : 64 * hh + 64, :],
                                    op=mult,
                                )
                    elif _stage_ge("norm"):
                        rb = rbp.tile([128, 512], f32, name="rb")
                        nc.vector.reciprocal(out=rb, in_=denb)
                        nc.vector.tensor_tensor(
                            out=zf[:, sj], in0=zden, in1=rb, op=mult
                        )
                    else:
                        nc.vector.tensor_copy(out=zf[:, sj], in_=zden)

          def _wo(b):
            x_sb, zfA, zfB = (S[b][k] for k in ("x_sb", "zfA", "zfB"))
            # ---- output projection + residual projection + bias
            po = pbig.tile([128, L], f32, name="pl")
            for j in range(2):
                sj = slice(512 * j, 512 * j + 512)
                nc.tensor.matmul(out=po[:, sj], lhsT=wo_sb[:, 0, :], rhs=zfA[:, sj],
                                 start=True, stop=False)
                nc.tensor.matmul(out=po[:, sj], lhsT=wo_sb[:, 1, :], rhs=zfB[:, sj],
                                 start=False, stop=False)
                nc.tensor.matmul(out=po[:, sj], lhsT=wo_sb[:, 2, :], rhs=x_sb[:, sj],
                                 start=False, stop=True)
            o_sb = outp.tile([128, L], f32, name="o_sb")
            nc.vector.tensor_scalar_add(out=o_sb, in0=po, scalar1=bout_sb[:, 0:1])
            nc.sync.dma_start(out=out_d[b], in_=o_sb)

          # Interleaved schedule: batch b+1's projections are emitted between
          # batch b's two attention head-groups so they execute on the PE FIFO
          # while ScalarE is still busy with batch b's exps; W_o is inline so
          # only the last batch's output tail is unoverlapped.
          for b in range(BLOC):
            _proj(b)
          for b in range(BLOC):
            _attn(b, 0)
            _attn(b, 1)
          for b in range(BLOC):
            _wo(b)

    _split_excess_waits(nc, mybir)
    nc.finalize()
    return nc


def get_nc():
    if "nc" not in _CACHE:
        _CACHE["nc"] = _build_nc()
    return _CACHE["nc"]


def prep_weights(w_qkv, b_qkv, w_o, b_o, w_res, b_res):
    w_qkv = np.asarray(w_qkv, np.float32)
    b_qkv = np.asarray(b_qkv, np.float32)
    w_o = np.asarray(w_o, np.float32)
    b_o = np.asarray(b_o, np.float32)
    w_res = np.asarray(w_res, np.float32)
    b_res = np.asarray(b_res, np.float32)

    d = np.arange(32)
    qrows = np.concatenate([96 * h + d for h in range(H)])        # (256,)
    krows = np.concatenate([96 * h + 32 + d for h in range(H)])
    vrows = np.concatenate([96 * h + 64 + d for h in range(H)])

    Wq = w_qkv[qrows] * SCALE                                     # (256, C)
    Wk = w_qkv[krows]
    wqk = np.stack([Wq[:128].T, Wq[128:].T, Wk[:128].T, Wk[128:].T], axis=1)
    bqk = np.stack([b_qkv[qrows[:128]], b_qkv[qrows[128:]]], axis=1) * SCALE
    wv = np.ascontiguousarray(w_qkv[vrows].T)                     # (C, 256)
    wo = np.stack([w_o[:, :128].T, w_o[:, 128:].T, w_res.T], axis=1)
    bv = b_qkv[vrows]
    bout = (b_o + b_res + w_o @ bv)[:, None]

    import ml_dtypes
    bf = ml_dtypes.bfloat16
    return {
        "wqk": np.ascontiguousarray(wqk, bf),
        "bqk": np.ascontiguousarray(bqk, np.float32),
        "wv": np.ascontiguousarray(wv, bf),
        "wo": np.ascontiguousarray(wo, bf),
        "bout": np.ascontiguousarray(bout, np.float32),
    }


def make_in_maps(x, weights):
    import ml_dtypes
    x = np.ascontiguousarray(np.asarray(x).astype(ml_dtypes.bfloat16))
    return [
        dict(x_sh=np.ascontiguousarray(x[BLOC * i : BLOC * i + BLOC]), **weights)
        for i in range(NCORES)
    ]


class Runner:
    """Persistent PJRT executable for the SPMD bass program (axon path).

    Mirrors concourse.bass2jax.run_bass_via_pjrt's multi-core branch, but keeps
    the jitted callable so repeated executions don't re-trace/re-compile —
    needed both for a fast kernel() and for timing loops in test.py.
    """

    def __init__(self, nc=None, donate=True):
        import jax
        import concourse.mybir as mybir
        from concourse import bass2jax
        from jax.experimental.shard_map import shard_map
        from jax.sharding import Mesh, PartitionSpec

        if nc is None:
            nc = get_nc()
        bass2jax.install_neuronx_cc_hook()

        in_names, out_names, out_avals = [], [], []
        partition_name = (
            nc.partition_id_tensor.name if nc.partition_id_tensor else None
        )
        for alloc in nc.m.functions[0].allocations:
            if not isinstance(alloc, mybir.MemoryLocationSet):
                continue
            name = alloc.memorylocations[0].name
            if alloc.kind == "ExternalInput":
                if name != partition_name:
                    in_names.append(name)
            elif alloc.kind == "ExternalOutput":
                shape = tuple(alloc.tensor_shape)
                dtype = mybir.dt.np(alloc.dtype)
                out_avals.append(jax.core.ShapedArray(shape, dtype))
                out_names.append(name)
        n_params = len(in_names)
        n_outs = len(out_avals)
        all_in_names = list(in_names) + list(out_names)
        if partition_name is not None:
            all_in_names.append(partition_name)
        self.in_names = in_names
        self.out_names = out_names
        self.out_avals = out_avals

        donate_idx = tuple(range(n_params, n_params + n_outs)) if donate else ()

        def _body(*args):
            operands = list(args)
            if partition_name is not None:
                operands.append(bass2jax.partition_id_tensor())
            outs = bass2jax._bass_exec_p.bind(
                *operands,
                out_avals=tuple(out_avals),
                in_names=tuple(all_in_names),
                out_names=tuple(out_names),
                lowering_input_output_aliases=(),
                sim_require_finite=True,
                sim_require_nnan=True,
                nc=nc,
            )
            return tuple(outs)

        devices = jax.devices()[:NCORES]
        assert len(devices) == NCORES
        mesh = Mesh(np.asarray(devices), ("core",))
        in_specs = (PartitionSpec("core"),) * (n_params + n_outs)
        out_specs = (PartitionSpec("core"),) * n_outs
        self.sharded = jax.jit(
            shard_map(_body, mesh=mesh, in_specs=in_specs, out_specs=out_specs,
                      check_rep=False),
            donate_argnums=donate_idx,
            keep_unused=True,
        )
        self.mesh = mesh

    def prep(self, in_maps):
        return [
            np.concatenate([np.asarray(m[name]) for m in in_maps], axis=0)
            for name in self.in_names
        ]

    def zeros(self):
        return [
            np.zeros((NCORES * a.shape[0], *a.shape[1:]), a.dtype)
            for a in self.out_avals
        ]

    def call_async(self, concat_in):
        return self.sharded(*concat_in, *self.zeros())

    def __call__(self, in_maps):
        outs = self.call_async(self.prep(in_maps))
        arr = np.asarray(outs[0])
        return arr.reshape(NCORES, *self.out_avals[0].shape)


def get_runner():
    if "runner" not in _CACHE:
        _CACHE["runner"] = Runner()
    return _CACHE["runner"]


def run(x, weights, **kw):
    runner = get_runner()
    per_core = runner(make_in_maps(x, weights))
    out = per_core.reshape(B, C, L)
    return out, None


def kernel(x, w_qkv, b_qkv, w_o, b_o, w_res, b_res):
    weights = prep_weights(w_qkv, b_qkv, w_o, b_o, w_res, b_res)
    out, _ = run(x, weights)
    return out

